# revision 15
# baseline (speedup 1.0000x reference)
"""Modulated deformable conv (DFConv2d) Trainium2 Bass kernel, v3.

Problem (hardcoded): x [4,256,64,64] f32; w_off [27,256,3,3]; b_off [27];
w_conv [256,256,3,3]; out [4,256,64,64].  K=3, pad=1, stride=1, dil=1.

Sharding: 8 cores = batch(4) x spatial-half(2).  Each core computes
out[b, :, s*32:(s+1)*32, :] (2048 output positions), pipelined in 4
chunks (cq) of 512 positions.

v3 design (vs v1 baseline at 220us):
  * The gather is bound by SWDGE descriptor-gen on gpsimd: 994ns fixed per
    indirect_dma_start call, one offset per partition (multi-offset calls
    are not supported by the deployed ucode - verified empirically), so
    2048 pos x 9 taps / 128 = 144 calls = ~150us of Pool time is the hard
    floor.  v3 makes Pool a pure descriptor-gen engine running at ~100%
    for the whole kernel and fits everything else underneath:
  * int8 gather table on a zero-padded 66x66 grid.  Halves gather DMA
    bytes (103us -> 52us) and removes all validity/edge-remap vector ops:
    out-of-bounds corners read genuine zeros from the pad; positions whose
    corner window leaves the padded grid have an exactly-zero true sample
    and are killed via one validity multiply folded into the mask (which
    also absorbs the int8 dequant scale).
  * The 576 corner-MAC ops ([128pos, 256ch], per-partition scalar weights;
    no DVE fast mode exists for ptr-scalar STT) are split DVE/ACT:
    corner00 on ACT (copy*scale init); corner01 mostly as ACT-mult +
    DVE-tensor-add pairs; corners 10/11 as DVE STT.  Sample-stage
    PSUM->SBUF copies on ACT.
  * Whole per-chunk chain (offset conv -> coords -> 36 gathers -> MAC ->
    transpose -> big matmul) is software-pipelined via tile pools, so the
    former 45us serial prologue overlaps the steady state.
"""

import numpy as np

import concourse.bass as bass
import concourse.bacc as bacc
import concourse.tile as tile
from concourse import mybir
from concourse.bass_utils import run_bass_kernel_spmd

F32 = mybir.dt.float32
F32R = mybir.dt.float32r
BF16 = mybir.dt.bfloat16
I32 = mybir.dt.int32
I8 = mybir.dt.int8
AF = mybir.ActivationFunctionType
OP = mybir.AluOpType

B, C, H, W, O = 4, 256, 64, 64, 256
K2 = 9
POS = 2048                 # positions per core (32 output rows)
NCQ = 4                    # position chunks per core
CQP = POS // NCQ           # 512 positions per chunk
NT = 66                    # padded grid side (64 + 2 pad)
TROWS = NT * NT            # 4356 gather-table rows
MAGIC = 12582912.0         # 1.5*2^23 float-floor magic


def build_program(debug=False, hw_gather_walk=True, reps=1):
    nc = bacc.Bacc("TRN2", target_bir_lowering=False)

    xs_t = nc.dram_tensor("xs", (C, 34 * 66), BF16, kind="ExternalInput")
    wof_t = nc.dram_tensor("wof", (C, K2 * 27), BF16, kind="ExternalInput")
    bof_t = nc.dram_tensor("bof", (27, 1), F32, kind="ExternalInput")
    w2_t = nc.dram_tensor("w2", (K2 * C, O), F32R, kind="ExternalInput")
    xq_t = nc.dram_tensor("xq", (TROWS, 4 * C), I8, kind="ExternalInput")
    byt_t = nc.dram_tensor("byt", (128, NCQ * 36), F32, kind="ExternalInput")
    bxt_t = nc.dram_tensor("bxt", (128, NCQ * 36), F32, kind="ExternalInput")
    ss_t = nc.dram_tensor("ss", (128, 1), F32, kind="ExternalInput")
    idnb_t = nc.dram_tensor("idnb", (128, 128), BF16, kind="ExternalInput")
    idn27_t = nc.dram_tensor("idn27", (27, 27), F32, kind="ExternalInput")
    out_t = nc.dram_tensor("out", (O, POS), F32, kind="ExternalOutput")

    with tile.TileContext(nc) as tc:
        with (
            tc.tile_pool(name="const", bufs=1) as constp,
            tc.tile_pool(name="coord", bufs=2) as coordp,
            tc.tile_pool(name="gbuf", bufs=8) as gbufp,
            tc.tile_pool(name="tmpb", bufs=4) as tmpp,
            tc.tile_pool(name="accb", bufs=3) as accp,
            tc.tile_pool(name="samp", bufs=2) as sampp,
            tc.tile_pool(name="outb", bufs=2) as outp,
            tc.tile_pool(name="ps_om", bufs=1, space="PSUM") as ps_om,
            tc.tile_pool(name="ps_t27", bufs=1, space="PSUM") as ps_t27,
            tc.tile_pool(name="ps_s", bufs=2, space="PSUM") as ps_s,
            tc.tile_pool(name="ps_out", bufs=2, space="PSUM") as ps_out,
        ):
            # ---- load constants (prologue-critical first, w2 last) ----
            xs_sb = []
            wof_sb = []
            for ct in range(2):
                t = constp.tile([128, 34 * 66], BF16, tag=f"xs{ct}", name=f"xs{ct}")
                nc.sync.dma_start(out=t[:], in_=xs_t[ct * 128:(ct + 1) * 128, :])
                xs_sb.append(t)
                t = constp.tile([128, K2 * 27], BF16, tag=f"wof{ct}", name=f"wof{ct}")
                nc.sync.dma_start(out=t[:], in_=wof_t[ct * 128:(ct + 1) * 128, :])
                wof_sb.append(t)
            byt = constp.tile([128, NCQ * 36], F32, tag="byt", name="byt")
            nc.sync.dma_start(out=byt[:], in_=byt_t[:])
            bxt = constp.tile([128, NCQ * 36], F32, tag="bxt", name="bxt")
            nc.sync.dma_start(out=bxt[:], in_=bxt_t[:])
            ss = constp.tile([128, 1], F32, tag="ss", name="ss")
            nc.sync.dma_start(out=ss[:], in_=ss_t[:])
            idn27 = constp.tile([27, 27], F32, tag="idn27", name="idn27")
            nc.sync.dma_start(out=idn27[:], in_=idn27_t[:])
            bof = constp.tile([27, 1], F32, tag="bof", name="bof")
            nc.sync.dma_start(out=bof[:], in_=bof_t[:])
            idnb = constp.tile([128, 128], BF16, tag="idnb", name="idnb")
            nc.sync.dma_start(out=idnb[:], in_=idnb_t[:])
            w2_sb = []
            for kt in range(18):
                t = constp.tile([128, O], F32R, tag=f"w2_{kt}", name=f"w2_{kt}")
                nc.sync.dma_start(out=t[:], in_=w2_t[kt * 128:(kt + 1) * 128, :])
                w2_sb.append(t)

            def head(cq):
                    # ---- offset conv: om[27, 512] for this chunk ----
                    pom = ps_om.tile([27, CQP], F32, tag="pom", name="pom")
                    first = True
                    for ct in range(2):
                        for k in range(K2):
                            ki, kj = k // 3, k % 3
                            rhs = (xs_sb[ct][:]
                                   .rearrange("p (r c) -> p r c", r=34)
                                   [:, cq * 8 + ki: cq * 8 + ki + 8, kj: kj + 64])
                            nc.tensor.matmul(
                                pom[:],
                                lhsT=wof_sb[ct][:, k * 27:(k + 1) * 27],
                                rhs=rhs,
                                start=first,
                                stop=(k == K2 - 1 and ct == 1),
                            )
                            first = False
                    om = coordp.tile([27, CQP], F32, tag="om", name="om")
                    nc.scalar.activation(out=om[:], in_=pom[:],
                                         func=AF.Identity, bias=bof[:, 0:1],
                                         scale=1.0)

                    # ---- transpose to position-major omt[128, c4, 27] ----
                    omt = coordp.tile([128, 4, 27], F32, tag="omt", name="omt")
                    for c4 in range(4):
                        ptp = ps_t27.tile([128, 27], F32, tag="omtp", name="omtp")
                        nc.tensor.transpose(
                            out=ptp[:],
                            in_=om[:, c4 * 128:(c4 + 1) * 128],
                            identity=idn27[:],
                        )
                        nc.vector.tensor_copy(out=omt[:, c4, :], in_=ptp[:])

                    def sm(tag, dt=F32):
                        return coordp.tile([128, 36], dt, tag=tag, name=tag)

                    def v3(ap):
                        return ap.rearrange("p (k c4) -> p k c4", k=K2)

                    # ---- coords (all [128, 36], free = (k, c4)) ----
                    ys = sm("ys")
                    nc.vector.tensor_tensor(
                        out=v3(ys[:]), in0=omt[:, :, 0:18:2].rearrange(
                            "p c4 k -> p k c4"),
                        in1=v3(byt[:, cq * 36:(cq + 1) * 36]), op=OP.add)
                    xs_ = sm("xs_")
                    nc.vector.tensor_tensor(
                        out=v3(xs_[:]), in0=omt[:, :, 1:18:2].rearrange(
                            "p c4 k -> p k c4"),
                        in1=v3(bxt[:, cq * 36:(cq + 1) * 36]), op=OP.add)

                    def floorf(v, tagp):
                        r = sm(tagp + "_r")
                        nc.vector.tensor_scalar(out=r[:], in0=v[:],
                                                scalar1=MAGIC, scalar2=None,
                                                op0=OP.add)
                        nc.vector.tensor_scalar(out=r[:], in0=r[:],
                                                scalar1=MAGIC, scalar2=None,
                                                op0=OP.subtract)
                        corr = sm(tagp + "_c")
                        nc.vector.tensor_tensor(out=corr[:], in0=r[:],
                                                in1=v[:], op=OP.is_gt)
                        f = sm(tagp + "_f")
                        nc.vector.tensor_tensor(out=f[:], in0=r[:],
                                                in1=corr[:], op=OP.subtract)
                        frac = sm(tagp + "_fr")
                        nc.vector.tensor_tensor(out=frac[:], in0=v[:],
                                                in1=f[:], op=OP.subtract)
                        return f, frac

                    py, ly = floorf(ys, "fy")   # py = floor(y)+1 (pad baked)
                    px, lx = floorf(xs_, "fx")

                    pyc = sm("pyc")
                    nc.vector.tensor_scalar(out=pyc[:], in0=py[:], scalar1=0.0,
                                            scalar2=64.0, op0=OP.max, op1=OP.min)
                    pxc = sm("pxc")
                    nc.vector.tensor_scalar(out=pxc[:], in0=px[:], scalar1=0.0,
                                            scalar2=64.0, op0=OP.max, op1=OP.min)
                    vy = sm("vy")
                    nc.vector.tensor_tensor(out=vy[:], in0=pyc[:], in1=py[:],
                                            op=OP.is_equal)
                    vx = sm("vx")
                    nc.vector.tensor_tensor(out=vx[:], in0=pxc[:], in1=px[:],
                                            op=OP.is_equal)
                    vv = sm("vv")
                    nc.vector.tensor_tensor(out=vv[:], in0=vy[:], in1=vx[:],
                                            op=OP.mult)

                    # mask = sigmoid(logits) * validity * dequant scale
                    mk = sm("mk")
                    nc.scalar.activation(
                        out=v3(mk[:]),
                        in_=omt[:, :, 18:27].rearrange("p c4 k -> p k c4"),
                        func=AF.Sigmoid)
                    mv = sm("mv")
                    nc.vector.tensor_tensor(out=mv[:], in0=mk[:], in1=vv[:],
                                            op=OP.mult)
                    nc.vector.scalar_tensor_tensor(
                        out=mv[:], in0=mv[:], scalar=ss[:, 0:1], in1=mv[:],
                        op0=OP.mult, op1=OP.bypass)

                    # corner weights: cw[a][b] = wy_a * wx_b * mv
                    wx1m = sm("wx1m")
                    nc.vector.tensor_tensor(out=wx1m[:], in0=lx[:], in1=mv[:],
                                            op=OP.mult)
                    wx0m = sm("wx0m")
                    nc.vector.tensor_tensor(out=wx0m[:], in0=mv[:], in1=wx1m[:],
                                            op=OP.subtract)
                    wy0 = sm("wy0")
                    nc.vector.tensor_scalar(out=wy0[:], in0=ly[:], scalar1=-1.0,
                                            scalar2=1.0, op0=OP.mult, op1=OP.add)
                    cw = {}
                    for (a, wya) in ((0, wy0), (1, ly)):
                        for (b, wxb) in ((0, wx0m), (1, wx1m)):
                            t = sm(f"cw{a}{b}")
                            nc.vector.tensor_tensor(out=t[:], in0=wya[:],
                                                    in1=wxb[:], op=OP.mult)
                            cw[(a, b)] = t

                    # ---- gather index: row = pyc*66 + pxc (int32) ----
                    tfi = sm("tfi")
                    nc.vector.tensor_scalar(out=tfi[:], in0=pyc[:],
                                            scalar1=float(NT), scalar2=None,
                                            op0=OP.mult)
                    nc.vector.tensor_tensor(out=tfi[:], in0=tfi[:], in1=pxc[:],
                                            op=OP.add)
                    idxi = coordp.tile([128, 36], I32, tag="idxi", name="idxi")
                    nc.vector.tensor_copy(out=idxi[:], in_=tfi[:])

                    if debug:
                        for nm, t in [("ys", ys), ("xs_", xs_), ("py", py),
                                      ("px", px), ("ly", ly), ("lx", lx),
                                      ("mv", mv), ("tfi", tfi)]:
                            dt_ = nc.dram_tensor(f"dbg_{nm}_{cq}", (128, 36),
                                                 F32, kind="ExternalOutput")
                            nc.sync.dma_start(out=dt_[:], in_=t[:])
                        for (a, b), t in cw.items():
                            dt_ = nc.dram_tensor(f"dbg_cw{a}{b}_{cq}",
                                                 (128, 36), F32,
                                                 kind="ExternalOutput")
                            nc.sync.dma_start(out=dt_[:], in_=t[:])

                    return cw, idxi

            def body(cq, cw, idxi):
                    # big-matmul PSUM accumulators, fed per tap
                    po = [ps_out.tile([128, CQP], F32, tag=f"po{m}",
                                      name=f"po{m}") for m in range(2)]
                    # ---- per tap: 4 gathers + corner MAC + transpose ----
                    for k in range(K2):
                        acc = accp.tile([128, 4, C], BF16, tag="acc",
                                        name="acc")
                        for c4 in range(4):
                            col = k * 4 + c4
                            g = gbufp.tile([128, 4 * C], I8, tag="g", name="g")
                            nc.gpsimd.indirect_dma_start(
                                out=g[:], out_offset=None, in_=xq_t[:],
                                in_offset=bass.IndirectOffsetOnAxis(
                                    ap=idxi[:, col:col + 1], axis=0),
                            )
                            gs = [g[:, j * C:(j + 1) * C] for j in range(4)]
                            # corner order in row: (y0x0),(y0x1),(y1x0),(y1x1)
                            nc.scalar.activation(
                                out=acc[:, c4, :], in_=gs[0], func=AF.Copy,
                                scale=cw[(0, 0)][:, col:col + 1])
                            if col % 3 != 0:
                                # corner01 as ACT-mult + DVE add (load balance)
                                tmp = tmpp.tile([128, C], BF16, tag="tmp",
                                                name="tmp")
                                nc.scalar.activation(
                                    out=tmp[:], in_=gs[1], func=AF.Copy,
                                    scale=cw[(0, 1)][:, col:col + 1])
                                nc.vector.tensor_tensor(
                                    out=acc[:, c4, :], in0=acc[:, c4, :],
                                    in1=tmp[:], op=OP.add)
                            else:
                                nc.vector.scalar_tensor_tensor(
                                    out=acc[:, c4, :], in0=gs[1],
                                    scalar=cw[(0, 1)][:, col:col + 1],
                                    in1=acc[:, c4, :], op0=OP.mult, op1=OP.add)
                            nc.vector.scalar_tensor_tensor(
                                out=acc[:, c4, :], in0=gs[2],
                                scalar=cw[(1, 0)][:, col:col + 1],
                                in1=acc[:, c4, :], op0=OP.mult, op1=OP.add)
                            nc.vector.scalar_tensor_tensor(
                                out=acc[:, c4, :], in0=gs[3],
                                scalar=cw[(1, 1)][:, col:col + 1],
                                in1=acc[:, c4, :], op0=OP.mult, op1=OP.add)
                        # transpose [pos, ch] -> [ch, pos]
                        ps = ps_s.tile([128, 2, 4, 128], BF16, tag="ps",
                                       name="ps")
                        for ct in range(2):
                            for c4 in range(4):
                                nc.tensor.transpose(
                                    out=ps[:, ct, c4, :],
                                    in_=acc[:, c4, ct * 128:(ct + 1) * 128],
                                    identity=idnb[:],
                                )
                        st = sampp.tile([128, 2, 4, 128], BF16,
                                        tag="samp", name="samp")
                        nc.scalar.activation(
                            out=st[:].rearrange("p a b c -> p (a b c)"),
                            in_=ps[:].rearrange("p a b c -> p (a b c)"),
                            func=AF.Copy)
                        # feed this tap into both output-chunk accumulators
                        for m in range(2):
                            for ct in range(2):
                                nc.tensor.matmul(
                                    po[m][:],
                                    lhsT=w2_sb[k * 2 + ct][:, m * 128:
                                                           (m + 1) * 128],
                                    rhs=st[:, ct].rearrange("p a b -> p (a b)"),
                                    start=(k == 0 and ct == 0),
                                    stop=(k == K2 - 1 and ct == 1),
                                )

                    # ---- drain accumulators ----
                    for m in range(2):
                        osb = outp.tile([128, CQP], F32, tag="osb", name="osb")
                        nc.scalar.activation(out=osb[:], in_=po[m][:],
                                             func=AF.Copy)
                        nc.sync.dma_start(
                            out=out_t[m * 128:(m + 1) * 128,
                                      cq * CQP:(cq + 1) * CQP],
                            in_=osb[:],
                        )

            # software pipeline: head runs one chunk ahead of body so the
            # gather descriptor-gen stream on Pool never waits for coords
            for rep in range(reps):
                hd = {0: head(0)}
                for cq in range(NCQ):
                    if cq + 1 < NCQ:
                        hd[cq + 1] = head(cq + 1)
                    body(cq, *hd.pop(cq))

    nc.compile()
    return nc


def host_inputs(x, w_off, b_off, w_conv):
    """Build the 8 per-core input maps (numpy only, layout prep)."""
    import ml_dtypes
    x = np.asarray(x, np.float32)
    w_off = np.asarray(w_off, np.float32)
    b_off = np.asarray(b_off, np.float32)
    w_conv = np.asarray(w_conv, np.float32)

    xp = np.zeros((B, C, 66, 66), np.float32)
    xp[:, :, 1:65, 1:65] = x
    wof = np.ascontiguousarray(
        w_off.reshape(27, C, K2).transpose(1, 2, 0)).reshape(
            C, K2 * 27).astype(ml_dtypes.bfloat16)
    w2 = np.ascontiguousarray(
        w_conv.reshape(O, C, K2).transpose(2, 1, 0)).reshape(K2 * C, O)
    bof = b_off.reshape(27, 1).astype(np.float32)
    idnb = np.eye(128, dtype=ml_dtypes.bfloat16)
    idn27 = np.eye(27, dtype=np.float32)

    # int8 gather tables, one per batch image, on a 67x67 construction pad
    xq_b = []
    ss_b = []
    for b in range(B):
        s = float(np.abs(x[b]).max()) / 127.0
        q67 = np.zeros((67, 67, C), np.int8)
        q67[1:65, 1:65] = np.clip(
            np.rint(x[b].transpose(1, 2, 0) / s), -127, 127).astype(np.int8)
        xq = np.concatenate(
            [q67[:66, :66, None], q67[:66, 1:67, None],
             q67[1:67, :66, None], q67[1:67, 1:67, None]],
            axis=2).reshape(TROWS, 4 * C)
        xq_b.append(np.ascontiguousarray(xq))
        ss_b.append(np.full((128, 1), s, np.float32))

    in_maps = []
    p = np.arange(128)
    k = np.arange(K2)
    c4 = np.arange(4)
    cqv = np.arange(NCQ)
    for core in range(8):
        b, sh = core // 2, core % 2
        xs = np.ascontiguousarray(
            xp[b, :, sh * 32: sh * 32 + 34, :]).reshape(
                C, 34 * 66).astype(ml_dtypes.bfloat16)
        # pos = cq*512 + c4*128 + p ; row = pos//64 ; col = pos%64
        pos = (cqv[:, None, None, None] * 512 + c4[None, None, :, None] * 128
               + p[None, None, None, :])                    # [cq, 1, c4, p]
        pos = np.broadcast_to(pos, (NCQ, K2, 4, 128))
        row = sh * 32 + pos // 64
        colw = pos % 64
        # padded-grid base incl. +1 pad offset: floor(y)+1 = floor(y + base+1)
        byt = (row + (k[None, :, None, None] // 3)).astype(np.float32)
        bxt = (colw + (k[None, :, None, None] % 3)).astype(np.float32)
        byt = byt.transpose(3, 0, 1, 2).reshape(128, NCQ * 36)
        bxt = bxt.transpose(3, 0, 1, 2).reshape(128, NCQ * 36)
        in_maps.append({
            "xs": xs, "wof": wof, "bof": bof, "w2": w2, "xq": xq_b[b],
            "byt": np.ascontiguousarray(byt),
            "bxt": np.ascontiguousarray(bxt),
            "ss": ss_b[b], "idnb": idnb, "idn27": idn27,
        })
    return in_maps


_NC = None


def kernel(x, w_off, b_off, w_conv):
    global _NC
    if _NC is None:
        _NC = build_program()
    in_maps = host_inputs(x, w_off, b_off, w_conv)
    res = run_bass_kernel_spmd(_NC, in_maps, core_ids=list(range(8)))
    out = np.empty((B, O, H, W), np.float32)
    for core in range(8):
        b, sh = core // 2, core % 2
        out[b, :, sh * 32:(sh + 1) * 32, :] = res.results[core]["out"].reshape(
            O, 32, 64)
    return out


# revision 18
# speedup vs baseline: 1.0621x; 1.0621x over previous
"""Modulated deformable conv (DFConv2d) Trainium2 Bass kernel, v4.

Problem (hardcoded): x [4,256,64,64] f32; w_off [27,256,3,3]; b_off [27];
w_conv [256,256,3,3]; out [4,256,64,64].  K=3, pad=1, stride=1, dil=1.

Sharding: 8 cores = batch(4) x spatial-half(2).  Each core computes
out[b, :, s*32:(s+1)*32, :] (2048 output positions), pipelined in 4
chunks (cq) of 512 positions.

v4 design (vs v1 baseline at 220us):
  * The gather is bound by SWDGE descriptor-gen on gpsimd: 994ns fixed per
    indirect_dma_start call, one offset per partition (multi-offset calls
    and dma_gather are not supported by the deployed ucode - verified
    empirically), so 2048 pos x 9 taps / 128 = 144 calls ~= 150us of Pool
    time is the hard floor.  v4 makes Pool a pure descriptor-gen engine
    running gapless and fits everything else underneath it:
  * bf16 4-corner gather table on a zero-padded 66x66 grid.  The pad
    removes all validity/edge-remap vector ops: out-of-bounds corners read
    genuine zeros; positions whose corner window leaves the padded grid
    have an exactly-zero true sample and are killed via one validity
    multiply folded into the mask.
  * Corner combine + transpose fused into PE: for each (tap, 128-pos
    group), sampled^T[c,p] = sum_j g_j[p,c] * w_j[p] is computed as four
    PSUM-accumulated matmuls with diagonal rhs matrices diag(w_j).  The
    diagonals are built as identity*weight-ptr ops split across DVE (3/4)
    and ACT (1/4) at ~193/292ns each - far cheaper than the per-corner
    [128,256] MACs they replace (no DVE fast mode exists for those).
  * Big matmul accumulates progressively per tap, so the chunk drains
    ~3us after its last gather.
  * Whole per-chunk chain is software-pipelined (head = conv+coords runs
    one chunk ahead of body = gathers+PE+matmul); PE is kept continuously
    busy from t~0 (warm-up spins) so the first offset conv runs at full
    p-state.
"""

import numpy as np

import concourse.bass as bass
import concourse.bacc as bacc
import concourse.tile as tile
from concourse import mybir
from concourse.bass_utils import run_bass_kernel_spmd

F32 = mybir.dt.float32
F32R = mybir.dt.float32r
BF16 = mybir.dt.bfloat16
I32 = mybir.dt.int32
AF = mybir.ActivationFunctionType
OP = mybir.AluOpType

B, C, H, W, O = 4, 256, 64, 64, 256
K2 = 9
POS = 2048                 # positions per core (32 output rows)
NCQ = 4                    # position chunks per core
CQP = POS // NCQ           # 512 positions per chunk
NT = 66                    # padded grid side (64 + 2 pad)
TROWS = NT * NT            # 4356 gather-table rows
MAGIC = 12582912.0         # 1.5*2^23 float-floor magic


def build_program(debug=False, hw_gather_walk=True, reps=1):
    nc = bacc.Bacc("TRN2", target_bir_lowering=False)

    xs_t = nc.dram_tensor("xs", (C, 34 * 66), BF16, kind="ExternalInput")
    wof_t = nc.dram_tensor("wof", (C, K2 * 27), BF16, kind="ExternalInput")
    bof_t = nc.dram_tensor("bof", (27, 1), F32, kind="ExternalInput")
    w2_t = nc.dram_tensor("w2", (K2 * C, O), F32R, kind="ExternalInput")
    xq_t = nc.dram_tensor("xq", (TROWS, 4 * C), BF16, kind="ExternalInput")
    byt_t = nc.dram_tensor("byt", (128, NCQ * 36), F32, kind="ExternalInput")
    bxt_t = nc.dram_tensor("bxt", (128, NCQ * 36), F32, kind="ExternalInput")
    idnb_t = nc.dram_tensor("idnb", (128, 128), BF16, kind="ExternalInput")
    idn27_t = nc.dram_tensor("idn27", (27, 27), F32, kind="ExternalInput")
    out_t = nc.dram_tensor("out", (O, POS), F32, kind="ExternalOutput")

    with tile.TileContext(nc) as tc:
        with (
            tc.tile_pool(name="const", bufs=1) as constp,
            tc.tile_pool(name="coord", bufs=2) as coordp,
            tc.tile_pool(name="gbuf", bufs=8) as gbufp,
            tc.tile_pool(name="diag", bufs=10) as diagp,
            tc.tile_pool(name="samp", bufs=3) as sampp,
            tc.tile_pool(name="outb", bufs=2) as outp,
            tc.tile_pool(name="ps_om", bufs=1, space="PSUM") as ps_om,
            tc.tile_pool(name="ps_t27", bufs=1, space="PSUM") as ps_t27,
            tc.tile_pool(name="ps_s", bufs=4, space="PSUM") as ps_s,
            tc.tile_pool(name="ps_out", bufs=1, space="PSUM") as ps_out,
        ):
            # ---- PE warm-up spins (p-state ramp) on a tiny zeroed tile ----
            wu = constp.tile([128, 64], BF16, tag="wu", name="wu")
            nc.vector.memset(wu[:], 0.0)
            wups = ps_t27.tile([64, 64], F32, tag="omtp", name="wups")
            for i in range(40):
                nc.tensor.matmul(wups[:], lhsT=wu[:], rhs=wu[:],
                                 start=True, stop=True)

            # ---- load constants (prologue-critical first, w2 last) ----
            xs_sb = []
            wof_sb = []
            for ct in range(2):
                t = constp.tile([128, 34 * 66], BF16, tag=f"xs{ct}", name=f"xs{ct}")
                # split the load: the first 10 rows unblock chunk 0's conv
                nc.sync.dma_start(
                    out=t[:].rearrange("p (r c) -> p r c", r=34)[:, 0:10, :],
                    in_=xs_t[ct * 128:(ct + 1) * 128, :].rearrange(
                        "p (r c) -> p r c", r=34)[:, 0:10, :])
                nc.sync.dma_start(
                    out=t[:].rearrange("p (r c) -> p r c", r=34)[:, 10:34, :],
                    in_=xs_t[ct * 128:(ct + 1) * 128, :].rearrange(
                        "p (r c) -> p r c", r=34)[:, 10:34, :])
                xs_sb.append(t)
                t = constp.tile([128, K2 * 27], BF16, tag=f"wof{ct}", name=f"wof{ct}")
                nc.sync.dma_start(out=t[:], in_=wof_t[ct * 128:(ct + 1) * 128, :])
                wof_sb.append(t)
            byt = constp.tile([128, NCQ * 36], F32, tag="byt", name="byt")
            nc.sync.dma_start(out=byt[:], in_=byt_t[:])
            bxt = constp.tile([128, NCQ * 36], F32, tag="bxt", name="bxt")
            nc.sync.dma_start(out=bxt[:], in_=bxt_t[:])
            idn27 = constp.tile([27, 27], F32, tag="idn27", name="idn27")
            nc.sync.dma_start(out=idn27[:], in_=idn27_t[:])
            bof = constp.tile([27, 1], F32, tag="bof", name="bof")
            nc.sync.dma_start(out=bof[:], in_=bof_t[:])
            idnb = constp.tile([128, 128], BF16, tag="idnb", name="idnb")
            nc.sync.dma_start(out=idnb[:], in_=idnb_t[:])
            w2_sb = []
            for kt in range(18):
                t = constp.tile([128, O], F32R, tag=f"w2_{kt}", name=f"w2_{kt}")
                nc.sync.dma_start(out=t[:], in_=w2_t[kt * 128:(kt + 1) * 128, :])
                w2_sb.append(t)

            def head(cq):
                    # ---- offset conv: om[27, 512] for this chunk ----
                    pom = ps_om.tile([27, CQP], F32, tag="pom", name="pom")
                    first = True
                    for ct in range(2):
                        for k in range(K2):
                            ki, kj = k // 3, k % 3
                            rhs = (xs_sb[ct][:]
                                   .rearrange("p (r c) -> p r c", r=34)
                                   [:, cq * 8 + ki: cq * 8 + ki + 8, kj: kj + 64])
                            nc.tensor.matmul(
                                pom[:],
                                lhsT=wof_sb[ct][:, k * 27:(k + 1) * 27],
                                rhs=rhs,
                                start=first,
                                stop=(k == K2 - 1 and ct == 1),
                            )
                            first = False
                    om = coordp.tile([27, CQP], F32, tag="om", name="om")
                    nc.scalar.activation(out=om[:], in_=pom[:],
                                         func=AF.Identity, bias=bof[:, 0:1],
                                         scale=1.0)

                    # ---- transpose to position-major omt[128, c4, 27] ----
                    omt = coordp.tile([128, 4, 27], F32, tag="omt", name="omt")
                    for c4 in range(4):
                        ptp = ps_t27.tile([128, 27], F32, tag="omtp", name="omtp")
                        nc.tensor.transpose(
                            out=ptp[:],
                            in_=om[:, c4 * 128:(c4 + 1) * 128],
                            identity=idn27[:],
                        )
                        nc.vector.tensor_copy(out=omt[:, c4, :], in_=ptp[:])

                    def sm(tag, dt=F32):
                        return coordp.tile([128, 36], dt, tag=tag, name=tag)

                    def v3(ap):
                        return ap.rearrange("p (k c4) -> p k c4", k=K2)

                    # ---- coords (all [128, 36], free = (k, c4)) ----
                    ys = sm("ys")
                    nc.vector.tensor_tensor(
                        out=v3(ys[:]), in0=omt[:, :, 0:18:2].rearrange(
                            "p c4 k -> p k c4"),
                        in1=v3(byt[:, cq * 36:(cq + 1) * 36]), op=OP.add)
                    xs_ = sm("xs_")
                    nc.vector.tensor_tensor(
                        out=v3(xs_[:]), in0=omt[:, :, 1:18:2].rearrange(
                            "p c4 k -> p k c4"),
                        in1=v3(bxt[:, cq * 36:(cq + 1) * 36]), op=OP.add)

                    def floorf(v, tagp):
                        r = sm(tagp + "_r")
                        nc.vector.tensor_scalar(out=r[:], in0=v[:],
                                                scalar1=MAGIC, scalar2=None,
                                                op0=OP.add)
                        nc.vector.tensor_scalar(out=r[:], in0=r[:],
                                                scalar1=MAGIC, scalar2=None,
                                                op0=OP.subtract)
                        corr = sm(tagp + "_c")
                        nc.vector.tensor_tensor(out=corr[:], in0=r[:],
                                                in1=v[:], op=OP.is_gt)
                        f = sm(tagp + "_f")
                        nc.vector.tensor_tensor(out=f[:], in0=r[:],
                                                in1=corr[:], op=OP.subtract)
                        frac = sm(tagp + "_fr")
                        nc.vector.tensor_tensor(out=frac[:], in0=v[:],
                                                in1=f[:], op=OP.subtract)
                        return f, frac

                    py, ly = floorf(ys, "fy")   # py = floor(y)+1 (pad baked)
                    px, lx = floorf(xs_, "fx")

                    pyc = sm("pyc")
                    nc.vector.tensor_scalar(out=pyc[:], in0=py[:], scalar1=0.0,
                                            scalar2=64.0, op0=OP.max, op1=OP.min)
                    pxc = sm("pxc")
                    nc.vector.tensor_scalar(out=pxc[:], in0=px[:], scalar1=0.0,
                                            scalar2=64.0, op0=OP.max, op1=OP.min)
                    vy = sm("vy")
                    nc.vector.tensor_tensor(out=vy[:], in0=pyc[:], in1=py[:],
                                            op=OP.is_equal)
                    vx = sm("vx")
                    nc.vector.tensor_tensor(out=vx[:], in0=pxc[:], in1=px[:],
                                            op=OP.is_equal)
                    vv = sm("vv")
                    nc.vector.tensor_tensor(out=vv[:], in0=vy[:], in1=vx[:],
                                            op=OP.mult)

                    # mask = sigmoid(logits) * validity
                    mk = sm("mk")
                    nc.scalar.activation(
                        out=v3(mk[:]),
                        in_=omt[:, :, 18:27].rearrange("p c4 k -> p k c4"),
                        func=AF.Sigmoid)
                    mv = sm("mv")
                    nc.vector.tensor_tensor(out=mv[:], in0=mk[:], in1=vv[:],
                                            op=OP.mult)

                    # corner weights: cw[a][b] = wy_a * wx_b * mv
                    wx1m = sm("wx1m")
                    nc.vector.tensor_tensor(out=wx1m[:], in0=lx[:], in1=mv[:],
                                            op=OP.mult)
                    wx0m = sm("wx0m")
                    nc.vector.tensor_tensor(out=wx0m[:], in0=mv[:], in1=wx1m[:],
                                            op=OP.subtract)
                    wy0 = sm("wy0")
                    nc.vector.tensor_scalar(out=wy0[:], in0=ly[:], scalar1=-1.0,
                                            scalar2=1.0, op0=OP.mult, op1=OP.add)
                    cw = {}
                    for (a, wya) in ((0, wy0), (1, ly)):
                        for (b, wxb) in ((0, wx0m), (1, wx1m)):
                            t = sm(f"cw{a}{b}")
                            nc.vector.tensor_tensor(out=t[:], in0=wya[:],
                                                    in1=wxb[:], op=OP.mult)
                            cw[(a, b)] = t

                    # ---- gather index: row = pyc*66 + pxc (int32) ----
                    tfi = sm("tfi")
                    nc.vector.tensor_scalar(out=tfi[:], in0=pyc[:],
                                            scalar1=float(NT), scalar2=None,
                                            op0=OP.mult)
                    nc.vector.tensor_tensor(out=tfi[:], in0=tfi[:], in1=pxc[:],
                                            op=OP.add)
                    idxi = coordp.tile([128, 36], I32, tag="idxi", name="idxi")
                    nc.vector.tensor_copy(out=idxi[:], in_=tfi[:])

                    if debug:
                        for nm, t in [("ys", ys), ("xs_", xs_), ("py", py),
                                      ("px", px), ("ly", ly), ("lx", lx),
                                      ("mv", mv), ("tfi", tfi)]:
                            dt_ = nc.dram_tensor(f"dbg_{nm}_{cq}", (128, 36),
                                                 F32, kind="ExternalOutput")
                            nc.sync.dma_start(out=dt_[:], in_=t[:])
                        for (a, b), t in cw.items():
                            dt_ = nc.dram_tensor(f"dbg_cw{a}{b}_{cq}",
                                                 (128, 36), F32,
                                                 kind="ExternalOutput")
                            nc.sync.dma_start(out=dt_[:], in_=t[:])

                    return cw, idxi

            CWO = [(0, 0), (0, 1), (1, 0), (1, 1)]  # corner order in xq row

            def body(cq, cw, idxi):
                    # big-matmul PSUM accumulators, fed per tap
                    po = [ps_out.tile([128, CQP], F32, tag=f"po{m}",
                                      name=f"po{m}") for m in range(2)]
                    for k in range(K2):
                        # gathers + diagonal weight mats for the 4 pos-groups
                        gs = []
                        dg = []
                        for c4 in range(4):
                            col = k * 4 + c4
                            g = gbufp.tile([128, 4 * C], BF16, tag="g", name="g")
                            nc.gpsimd.indirect_dma_start(
                                out=g[:], out_offset=None, in_=xq_t[:],
                                in_offset=bass.IndirectOffsetOnAxis(
                                    ap=idxi[:, col:col + 1], axis=0),
                            )
                            gs.append(g)
                            ds = []
                            for j in range(4):
                                d = diagp.tile([128, 128], BF16, tag="dg",
                                               name="dg")
                                scal = cw[CWO[j]][:, col:col + 1]
                                if j == 3:
                                    nc.scalar.activation(
                                        out=d[:], in_=idnb[:], func=AF.Copy,
                                        scale=scal)
                                else:
                                    nc.vector.scalar_tensor_tensor(
                                        out=d[:], in0=idnb[:], scalar=scal,
                                        in1=idnb[:], op0=OP.mult,
                                        op1=OP.bypass)
                                ds.append(d)
                            dg.append(ds)
                        # fused corner-combine + transpose on PE:
                        # ps[ct][c, c4, p] = sum_j g[c4][p, j*256+ct*128+c] * w_j[p]
                        for ct in range(2):
                            ps = ps_s.tile([128, 4, 128], F32, tag="ps",
                                           name="ps")
                            for c4 in range(4):
                                for j in range(4):
                                    nc.tensor.matmul(
                                        ps[:, c4, :],
                                        lhsT=gs[c4][:, j * C + ct * 128:
                                                    j * C + ct * 128 + 128],
                                        rhs=dg[c4][j][:],
                                        start=(j == 0),
                                        stop=(j == 3),
                                    )
                            st = sampp.tile([128, 4, 128], F32R, tag="samp",
                                            name="samp")
                            nc.scalar.activation(
                                out=st[:].rearrange("p a b -> p (a b)"),
                                in_=ps[:].rearrange("p a b -> p (a b)"),
                                func=AF.Copy)
                            # feed into both output-chunk accumulators
                            for m in range(2):
                                nc.tensor.matmul(
                                    po[m][:],
                                    lhsT=w2_sb[k * 2 + ct][:, m * 128:
                                                           (m + 1) * 128],
                                    rhs=st[:].rearrange("p a b -> p (a b)"),
                                    start=(k == 0 and ct == 0),
                                    stop=(k == K2 - 1 and ct == 1),
                                )

                    # ---- drain accumulators ----
                    for m in range(2):
                        osb = outp.tile([128, CQP], F32, tag="osb", name="osb")
                        nc.scalar.activation(out=osb[:], in_=po[m][:],
                                             func=AF.Copy)
                        nc.sync.dma_start(
                            out=out_t[m * 128:(m + 1) * 128,
                                      cq * CQP:(cq + 1) * CQP],
                            in_=osb[:],
                        )

            # software pipeline: head runs one chunk ahead of body so the
            # gather descriptor-gen stream on Pool never waits for coords
            for rep in range(reps):
                hd = {0: head(0)}
                for cq in range(NCQ):
                    if cq + 1 < NCQ:
                        hd[cq + 1] = head(cq + 1)
                    body(cq, *hd.pop(cq))

    nc.compile()
    return nc


def host_inputs(x, w_off, b_off, w_conv):
    """Build the 8 per-core input maps (numpy only, layout prep)."""
    import ml_dtypes
    x = np.asarray(x, np.float32)
    w_off = np.asarray(w_off, np.float32)
    b_off = np.asarray(b_off, np.float32)
    w_conv = np.asarray(w_conv, np.float32)

    xp = np.zeros((B, C, 66, 66), np.float32)
    xp[:, :, 1:65, 1:65] = x
    wof = np.ascontiguousarray(
        w_off.reshape(27, C, K2).transpose(1, 2, 0)).reshape(
            C, K2 * 27).astype(ml_dtypes.bfloat16)
    w2 = np.ascontiguousarray(
        w_conv.reshape(O, C, K2).transpose(2, 1, 0)).reshape(K2 * C, O)
    bof = b_off.reshape(27, 1).astype(np.float32)
    idnb = np.eye(128, dtype=ml_dtypes.bfloat16)
    idn27 = np.eye(27, dtype=np.float32)

    # bf16 4-corner gather tables, one per batch image (67x67 build pad)
    xq_b = []
    for b in range(B):
        q67 = np.zeros((67, 67, C), ml_dtypes.bfloat16)
        q67[1:65, 1:65] = x[b].transpose(1, 2, 0)
        xq = np.concatenate(
            [q67[:66, :66, None], q67[:66, 1:67, None],
             q67[1:67, :66, None], q67[1:67, 1:67, None]],
            axis=2).reshape(TROWS, 4 * C)
        xq_b.append(np.ascontiguousarray(xq))

    in_maps = []
    p = np.arange(128)
    k = np.arange(K2)
    c4 = np.arange(4)
    cqv = np.arange(NCQ)
    for core in range(8):
        b, sh = core // 2, core % 2
        xs = np.ascontiguousarray(
            xp[b, :, sh * 32: sh * 32 + 34, :]).reshape(
                C, 34 * 66).astype(ml_dtypes.bfloat16)
        # pos = cq*512 + c4*128 + p ; row = pos//64 ; col = pos%64
        pos = (cqv[:, None, None, None] * 512 + c4[None, None, :, None] * 128
               + p[None, None, None, :])                    # [cq, 1, c4, p]
        pos = np.broadcast_to(pos, (NCQ, K2, 4, 128))
        row = sh * 32 + pos // 64
        colw = pos % 64
        # padded-grid base incl. +1 pad offset: floor(y)+1 = floor(y + base+1)
        byt = (row + (k[None, :, None, None] // 3)).astype(np.float32)
        bxt = (colw + (k[None, :, None, None] % 3)).astype(np.float32)
        byt = byt.transpose(3, 0, 1, 2).reshape(128, NCQ * 36)
        bxt = bxt.transpose(3, 0, 1, 2).reshape(128, NCQ * 36)
        in_maps.append({
            "xs": xs, "wof": wof, "bof": bof, "w2": w2, "xq": xq_b[b],
            "byt": np.ascontiguousarray(byt),
            "bxt": np.ascontiguousarray(bxt),
            "idnb": idnb, "idn27": idn27,
        })
    return in_maps


_NC = None


def kernel(x, w_off, b_off, w_conv):
    global _NC
    if _NC is None:
        _NC = build_program()
    in_maps = host_inputs(x, w_off, b_off, w_conv)
    res = run_bass_kernel_spmd(_NC, in_maps, core_ids=list(range(8)))
    out = np.empty((B, O, H, W), np.float32)
    for core in range(8):
        b, sh = core // 2, core % 2
        out[b, :, sh * 32:(sh + 1) * 32, :] = res.results[core]["out"].reshape(
            O, 32, 64)
    return out
